# revision 18
# baseline (speedup 1.0000x reference)
"""Trainium2 Bass kernel for nn_BPDecoder: logits = 1 - exp(-exp(sum_i R_i*||Z_i||^2)).

Strategy (8-core SPMD, row-sharded, fp8 wire, quad-fused squares):
  - Pad N=500000 rows to 8 * 63488; core k takes rows [k*63488, (k+1)*63488).
  - Host scales Z by 512 -> fp8 e4m3 and pre-permutes into the on-device
    slab-major layout (within a slab of m tiles, partition p owns m*16
    consecutive rows) so every slab DMA is [128 x m*2KB] contiguous runs.
    Slabs alternate between the sync HWDGE ring and the gpsimd SWDGE ring
    (descriptor issuance on Pool does NOT contend with DVE, unlike Pool
    tensor ops which slow DVE 2.3->3.9us/tile).
  - Squares fp8 -> bf16, ONE instruction per slab (quad fusion amortizes
    the per-instruction overhead): ACT 17 tiles (~1.8us/tile), DVE 14
    (~2.2us/tile).  fp8 outputs measure 30-40% slower on both engines, so
    bf16 out + plain bf16 matmuls beat the fp8 DoubleRow path end to end.
  - PE: per tile 4 matmuls [128,16]x[128,512] with bf16 R stationary into
    one PSUM accumulation group accb[16, 2048] f32; dense burst feed keeps
    the PE HAM clock warm (2.4 GHz).  Host extracts q'==q diagonal blocks.
  - Final scalar: host sums diagonals, /512^2, applies 1 - exp(-exp(s)).
"""

import sys

sys.path.insert(0, "/opt/trn_rl_repo")


# The agent image lacks antenv.axon_hooks; recreate it so trace=True works
# (bass_utils imports it lazily for NTFF profiling under axon).
def _install_ntff_hook_shim():
    import types
    if "antenv.axon_hooks" in sys.modules:
        return
    mod = types.ModuleType("antenv.axon_hooks")
    state = {"hook": None}
    mod.set_axon_ntff_profile_hook = lambda h: state.__setitem__("hook", h)
    mod.get_axon_ntff_profile_hook = lambda: state["hook"]
    sys.modules["antenv.axon_hooks"] = mod
    try:
        sys.path.insert(0, "/root/.axon_site")
        from trn_agent_boot.trn_boot import _ntff_profile_via_ctypes
        state["hook"] = _ntff_profile_via_ctypes("/opt/axon/libaxon_pjrt.so")
    except Exception:
        pass


_install_ntff_hook_shim()

import numpy as np

import concourse.bass as bass
import concourse.bacc as bacc
import concourse.mybir as mybir
from concourse.tile import TileContext
from concourse.bass_utils import run_bass_kernel_spmd

P = 128          # SBUF partitions
D = 128          # row length (feature dim)
Q = 16           # rows per partition per tile
FREE = Q * D     # free elems per tile = 2048
T = 31           # tiles per core
NC_ROWS = T * P * Q   # 63488 rows per core
N_CORES = 8
N_FULL = 500000

Z_DT = mybir.dt.float8e4
RB_DT = mybir.dt.bfloat16
S_DT = mybir.dt.bfloat16

Z_SCALE_IN = 512.0

# slab layout: (n_tiles, engine); one square instruction per slab
SLABS = [
    (2, "dve"),
    (4, "act"),
    (4, "dve"),
    (4, "act"),
    (4, "dve"),
    (4, "act"),
    (4, "dve"),
    (4, "act"),
    (1, "act"),
]
assert sum(m for m, _ in SLABS) == T
SLAB_MAX = max(m for m, _ in SLABS)
SLAB_SIZES = [m for m, _ in SLABS]

_cache = {}


def _np_dt(dt):
    return mybir.dt.np(dt)


def _build():
    nc = bacc.Bacc(trn_type="TRN2")
    z = nc.declare_dram_parameter("z", [P, T, 4, 512], Z_DT, isOutput=False)
    rb = nc.declare_dram_parameter("rb", [P, T * Q], RB_DT, isOutput=False)
    outb = nc.declare_dram_parameter("outb", [Q, FREE], mybir.dt.float32,
                                     isOutput=True)

    slab_t0 = []
    pos = 0
    for m, _ in SLABS:
        slab_t0.append(pos)
        pos += m
    dma_rings = ["sync", "gpsimd"]

    with TileContext(nc) as tc:
        with (
            tc.tile_pool(name="zpool", bufs=4) as zpool,
            tc.tile_pool(name="spool", bufs=3) as spool,
            tc.tile_pool(name="singles", bufs=1) as singles,
            tc.tile_pool(name="ppool", bufs=1, space="PSUM") as ppool,
        ):
            rb_sb = singles.tile([P, T * Q], RB_DT)
            nc.sync.dma_start(out=rb_sb[:], in_=rb[:])
            # warm the ACT Square table during the first slab's DMA
            warm_sb = singles.tile([P, 2], mybir.dt.bfloat16)
            nc.scalar.square(warm_sb[:, 0:1], warm_sb[:, 1:2])

            accb = ppool.tile([Q, FREE], mybir.dt.float32, name="accb")

            for si, (m, eng) in enumerate(SLABS):
                t0 = slab_t0[si]
                z_sb = zpool.tile([P, SLAB_MAX, 4, 512], Z_DT, tag="z")
                ring = getattr(nc, dma_rings[si % 2])
                ring.dma_start(out=z_sb[:, :m], in_=z[:, t0:t0 + m])
                s_sb = spool.tile([P, SLAB_MAX, 4, 512], S_DT, tag="s")
                if eng == "act":
                    nc.scalar.square(s_sb[:, :m], z_sb[:, :m])
                else:
                    nc.vector.tensor_mul(s_sb[:, :m], z_sb[:, :m], z_sb[:, :m])
                for t in range(t0, t0 + m):
                    ti = t - t0
                    for sl in range(4):
                        nc.tensor.matmul(
                            accb[:, sl * 512:(sl + 1) * 512],
                            rb_sb[:, t * Q:(t + 1) * Q],
                            s_sb[:, ti, sl, :],
                            start=(t == 0),
                            stop=(t == T - 1),
                        )

            outb_sb = singles.tile([Q, FREE], mybir.dt.float32)
            for sl in range(4):
                copy_eng = nc.scalar.copy if sl % 2 == 0 else nc.vector.tensor_copy
                copy_eng(outb_sb[:, sl * 512:(sl + 1) * 512],
                         accb[:, sl * 512:(sl + 1) * 512])
            nc.sync.dma_start(out=outb[:], in_=outb_sb[:])
    nc.compile()
    return nc


def _get_nc():
    if "nc" not in _cache:
        _cache["nc"] = _build()
    return _cache["nc"]


def _shard(Z, R):
    np_z = _np_dt(Z_DT)
    ZP = np.zeros((N_CORES * NC_ROWS, D), dtype=np_z)
    ZP[:N_FULL] = (Z * np.float32(Z_SCALE_IN)).astype(np_z)
    RP = np.zeros((N_CORES * NC_ROWS,), dtype=np.float32)
    RP[:N_FULL] = R
    ZP = ZP.reshape(N_CORES, NC_ROWS, D)
    RP = RP.reshape(N_CORES, NC_ROWS)

    ZD = np.empty((N_CORES, P, T * FREE), dtype=np_z)
    RD = np.empty((N_CORES, P, T * Q), dtype=np.float32)
    pos = 0
    for m in SLAB_SIZES:
        t0 = pos
        zb = ZP[:, t0 * 2048:(t0 + m) * 2048].reshape(N_CORES, P, m * Q, D)
        ZD[:, :, t0 * FREE:(t0 + m) * FREE] = zb.reshape(N_CORES, P, m * FREE)
        rbk = RP[:, t0 * 2048:(t0 + m) * 2048].reshape(N_CORES, P, m * Q)
        RD[:, :, t0 * Q:(t0 + m) * Q] = rbk
        pos += m
    ZD = ZD.reshape(N_CORES, P, T, 4, 512)
    RB = RD.astype(_np_dt(RB_DT))
    return [{"z": ZD[k], "rb": RB[k]} for k in range(N_CORES)]


def _combine(results):
    s = 0.0
    idq = np.arange(Q)
    for res in results:
        Cb = np.asarray(res["outb"], dtype=np.float64).reshape(Q, Q, D)
        s += Cb[idq, idq, :].sum()
    s /= float(Z_SCALE_IN) ** 2
    lam = np.exp(s)
    logits = 1.0 - np.exp(-lam)
    return np.float32(logits)


def _run(Z, R, trace=False, tmpdir=None):
    nc = _get_nc()
    in_maps = _shard(np.asarray(Z), np.asarray(R))
    return run_bass_kernel_spmd(nc, in_maps, core_ids=list(range(N_CORES)),
                                trace=trace, tmpdir=tmpdir)


def kernel(Z, R):
    assert Z.shape == (N_FULL, D) and R.shape == (N_FULL,)
    out = _run(np.asarray(Z), np.asarray(R), trace=False)
    return _combine(out.results)
